# revision 43
# baseline (speedup 1.0000x reference)
"""Causal multi-head attention block (16 heads, dim 1024) on 8 TRN2 NeuronCores.

Sharding: tensor-parallel over heads - core c computes heads {2c, 2c+1}:
  q/k/v projections with the 128-column weight slices, causal attention,
  and a partial output projection with the matching 128 Wout rows.
Host sums the 8 partial outputs and adds the bias.

Design notes (per-chunk software pipeline + fp8 DoubleRow q/k):
  * The host supplies x PRE-TRANSPOSED and chunk-blocked in TWO copies:
    bf16 (for the v projection) and fp8e4m3 (for q/k), both laid out so
    each per-chunk DMA reads 8KB contiguous per partition (the flat
    [dim, b*n] layout produced ~600B packets and 130us of DMA active
    time).  Wq/Wk are prescaled x32 (their std 1/32 sits in e4m3's
    subnormal range) and quantized to fp8; the x1024 factor on the scores
    is folded into the exp scale.  Measured rel err 6.4e-3 vs 3.2e-3 all-
    bf16 (threshold 2e-2).
  * q/k projections run as fp8 DoubleRow matmuls (lhsT [128,2,128] /
    rhs [128,2,512], K_eff=256 per step, 2x PE throughput).  v (accuracy-
    critical) stays bf16, TOKEN-major -> vaug ([128 j, 65] per j-tile per
    head: 64 v cols + a ones col that makes the AV matmul also produce
    softmax denominators).
  * scores TRANSPOSED: dotsT[j,i] = kT.T @ qT per (j-tile, head), K=64 ->
    the two heads go to PE row groups (0,0)/(64,0) and run CONCURRENTLY
    into one [128, 1024] psum tile PER J-TILE from a double-buffered
    pool: scores(jt+1) write the other buffer while the ACT exp reads
    jt's, so the PE NEVER waits on ACT for scores.  The exp uses a
    strided [2, 512-off] AP (valid columns only); diagonal tiles are
    zeroed above the diagonal with one both-heads gpsimd affine_select.
  * AV matmuls lag 2 j-tiles behind their exp so the in-order PE reaches
    them after the exp has long finished.
  * Pipeline granularity is a (batch, 512-token chunk) unit: attention
    for unit u runs interleaved (credit-weighted generator round-robin)
    with projections for unit u+1, so the PE always has independent
    matmul work queued under the exp chain.  This kills the startup
    bubble and the ACT-bound tail the batch-level pipeline had (the PE
    also drops out of its 2.4GHz p-state after ~any stall, so gaps cost
    ~2x their length).
  * The output projection of chunk c is deferred (FIFO backlog, floor 4)
    into later chunks' jt loops as PE filler; the floor keeps entries
    whose normalize stt may still be in flight out of reach of the
    in-order PE queue.  The final unit drains the backlog (its slot has
    no projection partner to interleave).
  * Normalize chain (den copy -> partition-0-staged
    reciprocal_approx_fast -> gpsimd partition_broadcast -> stt) is
    software-pipelined across the two heads.
  * Bulk DMA stays on nc.sync; the dependency-free weight loads ride the
    scalar/gpsimd queues at startup so all transfers overlap.
"""
import numpy as np
import ml_dtypes
from contextlib import ExitStack

import concourse.bacc as bacc
import concourse.mybir as mybir
import concourse.tile as tile
import concourse.bass_utils as bass_utils

F32 = mybir.dt.float32
BF16 = mybir.dt.bfloat16
FP16 = mybir.dt.float16
FP8 = mybir.dt.float8e4

B = 4            # batches
T = 2048         # tokens per batch
DIM = 1024
NT = T // 128    # token tiles per batch (16)
KT = DIM // 128  # contraction tiles (8)
NCHUNK = T // 512  # 512-col i-chunks per batch (4)
SCALE = DIM ** -0.5  # 1/32 - NOTE: full dim, not head dim (matches reference)
QK_PRESCALE = 32.0   # Wq/Wk are x32 before fp8 quantization (their std is
                     # 1/32, below e4m3's normal range); folded out of the
                     # scores inside the exp scale

TRACE = False
LAST_EXEC_NS = None
LAST_TRACE = None
LAST_PROFILE = None
_CACHED = {}


def build_kernel():
    nc = bacc.Bacc("TRN2", target_bir_lowering=False, debug=False, num_devices=8)

    xT_d = nc.dram_tensor("xt", [128, B * NCHUNK * KT * 512], BF16,
                          kind="ExternalInput").ap()
    # fp8 copy of x + 32x-prescaled fp8 Wq/Wk for the DoubleRow q/k
    # projections (contraction pairs (kt2, s): feature = (2*kt2+s)*128+p)
    x8_d = nc.dram_tensor("x8", [128, B * NCHUNK * KT * 512], FP8,
                          kind="ExternalInput").ap()
    w8q_d = nc.dram_tensor("w8q", [128, KT * 128], FP8, kind="ExternalInput").ap()
    w8k_d = nc.dram_tensor("w8k", [128, KT * 128], FP8, kind="ExternalInput").ap()
    # wv is host-pre-arranged to the SBUF layout [128, KT*128]
    # (wX[p, kt*128+m] = W[kt*128+p, m]) so its DMA is fully contiguous
    wv_d = nc.dram_tensor("wv", [128, KT * 128], BF16, kind="ExternalInput").ap()
    wo_d = nc.dram_tensor("wo", [128, DIM], BF16, kind="ExternalInput").ap()
    out_d = nc.dram_tensor("out", [B * T, DIM], FP16, kind="ExternalOutput").ap()

    xsrc = xT_d.rearrange("p (u kt t) -> p u kt t", kt=KT, t=512)
    x8src = x8_d.rearrange("p (u k2 s t) -> p u k2 s t", k2=KT // 2, s=2, t=512)

    with tile.TileContext(nc) as tc, ExitStack() as ctx:
        cp = ctx.enter_context(tc.tile_pool(name="const", bufs=1))
        xT_p = ctx.enter_context(tc.tile_pool(name="xT", bufs=2))
        x8_p = ctx.enter_context(tc.tile_pool(name="x8", bufs=2))
        qT_p = ctx.enter_context(tc.tile_pool(name="qT", bufs=2))
        kT_p = ctx.enter_context(tc.tile_pool(name="kT", bufs=2))
        vaug_p = ctx.enter_context(tc.tile_pool(name="vaug", bufs=2))
        attnT_p = ctx.enter_context(tc.tile_pool(name="attnT", bufs=8))
        recip_p = ctx.enter_context(tc.tile_pool(name="recip", bufs=4))
        avsb_p = ctx.enter_context(tc.tile_pool(name="avsb", bufs=4))  # noqa: F841 - keeps the SBUF layout of the best-measured build
        rbc_p = ctx.enter_context(tc.tile_pool(name="rbc", bufs=2))
        outT_p = ctx.enter_context(tc.tile_pool(name="outT", bufs=2))
        osb_p = ctx.enter_context(tc.tile_pool(name="osb", bufs=3))
        mm_ps = ctx.enter_context(tc.tile_pool(name="mmps", bufs=2, space="PSUM"))
        dots_ps = ctx.enter_context(tc.tile_pool(name="dotsps", bufs=2, space="PSUM"))
        av_ps_p = ctx.enter_context(tc.tile_pool(name="avps", bufs=2, space="PSUM"))

        # ---- weights.  Only wq rides the sync DMA queue (ahead of the x
        # slices); wk/wv/wo are issued from the idle vector/gpsimd/scalar
        # queues so all four transfer in parallel at startup (the issuing
        # engine only blocks on a DMA's dependencies, and weights have
        # none) ----
        w8q_sb = cp.tile([128, KT * 128], FP8, tag="w8q")
        w8k_sb = cp.tile([128, KT * 128], FP8, tag="w8k")
        wv_sb = cp.tile([128, KT * 128], BF16, tag="wv")
        wo_sb = cp.tile([128, DIM], BF16, tag="wo")
        w8qv = w8q_sb[:].rearrange("p (k2 s m) -> p k2 s m", k2=KT // 2, s=2)
        w8kv = w8k_sb[:].rearrange("p (k2 s m) -> p k2 s m", k2=KT // 2, s=2)

        def dma_weights():
            nc.scalar.dma_start(w8q_sb[:], w8q_d)
            nc.scalar.dma_start(w8k_sb[:], w8k_d)
            nc.gpsimd.dma_start(wv_sb[:], wv_d)
            nc.gpsimd.dma_start(wo_sb[:], wo_d)

        # ---- constants ----
        ones32 = cp.tile([128, NT], F32, tag="ones32")
        nc.gpsimd.memset(ones32[:], 1.0)
        onesv = ones32[:].rearrange("p (u o) -> p u o", o=1)

        state = {}     # b -> (qT, kT, vaug, outT)
        xstate = {}    # b -> xTv view (allocated 2 slices ahead of use)
        deferred = []  # (outT, t0, tt) output-projection backlog

        def ensure_xT(b2):
            if b2 not in xstate:
                xT = xT_p.tile([128, NCHUNK * KT * 512], BF16, tag="xT",
                               name="xT")
                x8 = x8_p.tile([128, NCHUNK * KT * 512], FP8, tag="x8",
                               name="x8")
                xstate[b2] = (
                    xT[:].rearrange("p (c kt t) -> p c kt t", c=NCHUNK, kt=KT),
                    x8[:].rearrange("p (c k2 s t) -> p c k2 s t",
                                    c=NCHUNK, k2=KT // 2, s=2))

        def dma_slice_u(u):
            b2, c2 = divmod(u, NCHUNK)
            if b2 >= B:
                return
            ensure_xT(b2)
            xTv, x8v = xstate[b2]
            nc.sync.dma_start(x8v[:, c2], x8src[:, u])
            nc.sync.dma_start(xTv[:, c2, :, :], xsrc[:, u, :, :])

        def emit_oproj(outT, t0, tt):
            osb = osb_p.tile([128, DIM], FP16, tag="osb", name="osb")
            for half in (0, 1):
                po = mm_ps.tile([128, 512], F32, tag="mm", name="po")
                nc.tensor.matmul(po[:], outT[:, tt * 128:(tt + 1) * 128],
                                 wo_sb[:, half * 512:(half + 1) * 512],
                                 start=True, stop=True)
                nc.vector.tensor_copy(osb[:, half * 512:(half + 1) * 512],
                                      po[:])
            nc.sync.dma_start(out_d[t0 + tt * 128: t0 + (tt + 1) * 128, :],
                              osb[:])

        def p12_chunk_steps(b, ch):
            """xT DMA + q/k (feat-major) + v (token-major) for chunk ch of
            batch b."""
            u = b * NCHUNK + ch
            if ch == 0:
                ensure_xT(b)
                qT = qT_p.tile([128, T], BF16, tag="qT", name="qT")
                kTt = kT_p.tile([128, T], BF16, tag="kT", name="kT")
                vaug = vaug_p.tile([128, NT * 130], BF16, tag="vaug", name="vaug")
                outT = outT_p.tile([128, T], BF16, tag="outT", name="outT")
                state[b] = (qT, kTt, vaug, outT)
            qT, kTt, vaug, outT = state[b]
            xTv, x8v = xstate[b]

            # keep the x DMA two chunk-slots ahead of the projections
            # (cross-batch too, so a batch's first chunk is ready on arrival)
            if u == 0:
                dma_weights()
                nc.sync.dma_start(x8v[:, 0], x8src[:, 0])
                # bf16 slice 0 (only v-proj needs it) rides the scalar
                # queue, in parallel with x8 slice 0 on sync
                nc.scalar.dma_start(xTv[:, 0, :, :], xsrc[:, 0, :, :])
                dma_slice_u(1)
            dma_slice_u(u + 2)
            yield
            # q and k projections for this 512-token chunk: fp8 DoubleRow
            # matmuls (K_eff=256 per step, 2x PE throughput); yield
            # mid-accumulation so the PE work spreads across the
            # interleaved attention pair steps
            for w8v, dest in ((w8qv, qT), (w8kv, kTt)):
                pp = mm_ps.tile([128, 512], F32, tag="mm", name="pp")
                for k2 in range(KT // 2):
                    nc.tensor.matmul(pp[:], w8v[:, k2], x8v[:, ch, k2],
                                     start=(k2 == 0), stop=(k2 == KT // 2 - 1),
                                     perf_mode=mybir.MatmulPerfMode.DoubleRow)
                    if k2 == 1:
                        yield
                nc.vector.tensor_copy(dest[:, ch * 512:(ch + 1) * 512], pp[:])
                yield
            # v for the same 4 token-tiles, token-major
            vv = vaug[:].rearrange("p (jt c) -> p jt c", c=130)
            vp = mm_ps.tile([128, 512], F32, tag="mm", name="vp")
            for j in range(4):
                for kt in range(KT):
                    nc.tensor.matmul(
                        vp[:, j * 128:(j + 1) * 128],
                        xTv[:, ch, kt, j * 128:(j + 1) * 128],
                        wv_sb[:, kt * 128:(kt + 1) * 128],
                        start=(kt == 0), stop=(kt == KT - 1))
                if j < 3:
                    yield
            src = vp[:].rearrange("p (j c) -> p j c", j=4)
            nc.vector.tensor_copy(vv[:, 4 * ch:4 * ch + 4, 0:64], src[:, :, 0:64])
            nc.vector.tensor_copy(vv[:, 4 * ch:4 * ch + 4, 65:129], src[:, :, 64:128])
            # ones columns via DVE (gpsimd's in-order queue is slow on
            # semaphore ops and would delay the normalize broadcast)
            nc.vector.tensor_copy(vv[:, 4 * ch:4 * ch + 4, 64:65],
                                  onesv[:, 4 * ch:4 * ch + 4, :])
            nc.vector.tensor_copy(vv[:, 4 * ch:4 * ch + 4, 129:130],
                                  onesv[:, 4 * ch:4 * ch + 4, :])
            yield

        def att_chunk_steps(b, c, hold=4, drain=False):
            """Attention chunk c of batch b + deferred output projections."""
            t0 = b * T
            qT, kTt, vaug, outT = state[b]
            njt = 4 * (c + 1)
            avp = {h: av_ps_p.tile([65, 512], F32, tag="av", name=f"avp{h}")
                   for h in (0, 1)}

            def emit_av(pend):
                jt, off, at = pend
                for h in (0, 1):
                    nc.tensor.matmul(
                        avp[h][:, off:512],
                        vaug[:, jt * 130 + 65 * h: jt * 130 + 65 * h + 65],
                        at[:, h * 512 + off: (h + 1) * 512],
                        start=(jt == 0), stop=(jt == njt - 1))

            # one [128,1024] dots tile PER J-TILE from a double-buffered
            # pool: scores(jt+1) write the other buffer while exp(jt) reads,
            # so the PE NEVER waits on ACT for scores (no psum WAR).  The AV
            # matmuls lag 2 j-tiles so their exp is long done when the
            # in-order PE reaches them.
            pends = []
            popped = 0
            pop_target = 8 if drain else 4
            for jt in range(njt):
                off = max(512 * c, jt * 128) - 512 * c
                dps = dots_ps.tile([128, 1024], F32, tag="dots", name="dp")
                # h0/h1 hit disjoint PE row groups (rows 0-63 / 64-127) and
                # run concurrently
                for h in (0, 1):
                    nc.tensor.matmul(
                        dps[:, h * 512 + off: (h + 1) * 512],
                        kTt[64 * h:64 * h + 64, jt * 128:(jt + 1) * 128],
                        qT[64 * h:64 * h + 64, 512 * c + off:512 * (c + 1)],
                        start=True, stop=True)
                at = attnT_p.tile([128, 1024], BF16, tag="at", name="at")
                # exp only the valid columns of both heads (strided AP)
                dv = dps[:].rearrange("p (h i) -> p h i", h=2)
                atv = at[:].rearrange("p (h i) -> p h i", h=2)
                nc.scalar.activation(atv[:, :, off:512], dv[:, :, off:512],
                                     mybir.ActivationFunctionType.Exp,
                                     bias=0.0,
                                     scale=float(SCALE / QK_PRESCALE ** 2))
                if jt >= 4 * c:  # zero invalid (j > i) entries of the diag tile
                    # one op for both heads via the strided [2, 128] AP
                    # (halves the gpsimd op+semaphore count on the chunk tail)
                    nc.gpsimd.affine_select(
                        out=atv[:, :, off: off + 128],
                        in_=atv[:, :, off: off + 128],
                        compare_op=mybir.AluOpType.is_ge, fill=0.0,
                        base=0, pattern=[[0, 2], [1, 128]], channel_multiplier=-1)
                if len(pends) >= 2:
                    emit_av(pends.pop(0))
                # oproj backlog as PE filler, PACED evenly across the jt
                # loop (clustered pops left later jts with no filler under
                # the 1.3us exp and the PE dropped its p-state).  The >4
                # floor keeps the just-normalized chunk's entries (whose
                # stt may still be in flight - an oproj waiting on it would
                # block the in-order PE queue) out of reach; the drain unit
                # relaxes it once its own first jts are past.
                gate = hold if jt < 2 else min(hold, 0 if drain else hold)
                if (len(deferred) > gate
                        and (jt + 1) * pop_target >= (popped + 1) * njt):
                    emit_oproj(*deferred.pop(0))
                    popped += 1
                pends.append((jt, off, at))
                yield
            # chunk tail: reserved oprojs interleave with the trailing AVs
            while pends:
                if deferred:
                    emit_oproj(*deferred.pop(0))
                emit_av(pends.pop(0))
            yield
            # normalize, software-pipelined across the two heads so the
            # gpsimd broadcast of h0 overlaps the DVE recip of h1 (an
            # SBUF-staged variant freed the av psum banks earlier but its
            # +20us of DVE staging measured net-slower).  NOTE: den must be
            # staged to a partition-0 tile - the custom-DVE reciprocal
            # misbehaves on mismatched partition offsets.
            rc, rb = {}, {}
            for h in (0, 1):
                den = recip_p.tile([1, 512], F32, tag="den", name="den")
                nc.vector.tensor_copy(den[:], avp[h][64:65, :])
                rc[h] = recip_p.tile([1, 512], F32, tag="recip", name="rc")
                nc.vector.reciprocal_approx_fast(rc[h][:], den[:])
                if h == 0:
                    rb[0] = rbc_p.tile([64, 512], F32, tag="rbc", name="rb")
                    nc.gpsimd.partition_broadcast(rb[0][:], rc[0][:])
            rb[1] = rbc_p.tile([64, 512], F32, tag="rbc", name="rb")
            nc.gpsimd.partition_broadcast(rb[1][:], rc[1][:])
            for h in (0, 1):
                nc.vector.scalar_tensor_tensor(
                    outT[64 * h:64 * h + 64, c * 512:(c + 1) * 512],
                    avp[h][0:64, :], 1.0, rb[h][:],
                    op0=mybir.AluOpType.mult, op1=mybir.AluOpType.mult)
            deferred.extend((outT, t0, tt) for tt in range(4 * c, 4 * c + 4))
            yield

        def drive(gens):
            """Credit-weighted round-robin of (generator, weight) pairs: a
            generator advances ~weight steps per round, so both exhaust at
            the same time and the emission (= in-order engine queue order)
            keeps independent work spread between the dependency chains."""
            gens = [gw for gw in gens if gw is not None and gw[0] is not None]
            credit = [0.0] * len(gens)
            alive = [True] * len(gens)
            while any(alive):
                for i, (g, w) in enumerate(gens):
                    if not alive[i]:
                        continue
                    credit[i] += w
                    while credit[i] >= 1.0 and alive[i]:
                        credit[i] -= 1.0
                        try:
                            next(g)
                        except StopIteration:
                            alive[i] = False

        # oproj backlog floor per attention unit: steady 4 (defers each
        # chunk's oproj ~2 chunks), hoard through the penultimate units so
        # the ACT-bound final unit (no p12 partner) has PE filler, then
        # drain it there (hold 0)
        DRAIN_UNIT = (B - 1, NCHUNK - 1)  # last unit: ACT-bound, no p12
        # partner - drain the whole oproj backlog through its jt loop
        P12_STEPS = 9.0
        units = [(b, ch) for b in range(B) for ch in range(NCHUNK)]
        prev = None
        for i, (b, ch) in enumerate(units):
            att = (att_chunk_steps(*prev, drain=(prev == DRAIN_UNIT))
                   if prev is not None else None)
            att_steps = 4 * (prev[1] + 1) + 2 if prev is not None else 1
            drive([(att, 1.0) if att is not None else None,
                   (p12_chunk_steps(b, ch),
                    P12_STEPS / att_steps if att is not None else 4.0)])
            prev = (b, ch)
        drive([(att_chunk_steps(*prev, drain=(prev == DRAIN_UNIT)), 1.0)])
        while deferred:
            emit_oproj(*deferred.pop(0))

    nc.compile()
    return nc


def kernel(x, Wq, Wkv, Wout, bout):
    """Full inputs -> full output. Shards across 8 NeuronCores internally."""
    global LAST_EXEC_NS, LAST_TRACE
    if "nc" not in _CACHED:
        _CACHED["nc"] = build_kernel()
    nc = _CACHED["nc"]

    hdt = ml_dtypes.bfloat16
    f8 = ml_dtypes.float8_e4m3
    xf = np.asarray(x, dtype=np.float32).reshape(B, NCHUNK, 512, DIM)
    # [128, B, NCHUNK, KT, 512]: per-(chunk,partition) rows are 8KB
    # contiguous in DRAM so the per-chunk DMA moves full-size packets
    xTf = (xf.transpose(3, 0, 1, 2)         # [DIM, B, NCHUNK, 512]
             .reshape(KT, 128, B, NCHUNK, 512)
             .transpose(1, 2, 3, 0, 4))
    xT = np.ascontiguousarray(xTf).astype(hdt).reshape(128, -1)
    # fp8 copy with the kt dim regrouped into DoubleRow (kt2, s) pairs:
    # feature = (2*kt2+s)*128+p, laid out [p, b, ch, kt2, s, t]
    x8 = np.ascontiguousarray(xTf).astype(f8).reshape(128, -1)
    Wq = np.asarray(Wq, dtype=np.float32)
    Wkv = np.asarray(Wkv, dtype=np.float32)
    Wout = np.asarray(Wout, dtype=np.float32).astype(hdt)
    bout = np.asarray(bout, dtype=np.float32)

    def wlayout(w):  # [DIM, 128] -> [128, KT*128] SBUF layout for clean DMA
        return np.ascontiguousarray(
            w.reshape(KT, 128, 128).transpose(1, 0, 2).reshape(128, KT * 128))

    def wlayout8(w):  # same, x32 prescale, fp8
        return wlayout(np.asarray(w) * QK_PRESCALE).astype(f8)

    in_maps = []
    for c in range(8):
        s = slice(128 * c, 128 * (c + 1))
        in_maps.append({
            "xt": xT,
            "x8": x8,
            "w8q": wlayout8(Wq[:, s]),
            "w8k": wlayout8(Wkv[:, :DIM][:, s]),
            "wv": wlayout(Wkv[:, DIM:][:, s].astype(hdt)),
            "wo": np.ascontiguousarray(Wout[s, :]),
        })

    res = bass_utils.run_bass_kernel_spmd(nc, in_maps, core_ids=list(range(8)),
                                          trace=TRACE)
    if TRACE:
        LAST_EXEC_NS = res.exec_time_ns
        LAST_TRACE = res.instructions_and_trace
        globals()["LAST_PROFILE"] = getattr(res, "profile_json", None)
    acc = res.results[0]["out"].astype(np.float64)
    for c in range(1, 8):
        acc += res.results[c]["out"]
    out = (acc + bout.astype(np.float64)).astype(np.float32)
    return out.reshape(B, T, DIM)


# revision 44
# speedup vs baseline: 1.0351x; 1.0351x over previous
"""Causal multi-head attention block (16 heads, dim 1024) on 8 TRN2 NeuronCores.

Sharding: tensor-parallel over heads - core c computes heads {2c, 2c+1}:
  q/k/v projections with the 128-column weight slices, causal attention,
  and a partial output projection with the matching 128 Wout rows.
Host sums the 8 partial outputs and adds the bias.

Design notes (per-chunk software pipeline + fp8 DoubleRow q/k):
  * The host supplies x PRE-TRANSPOSED and chunk-blocked in TWO copies:
    bf16 (for the v projection) and fp8e4m3 (for q/k), both laid out so
    each per-chunk DMA reads 8KB contiguous per partition (the flat
    [dim, b*n] layout produced ~600B packets and 130us of DMA active
    time).  Wq/Wk are prescaled x32 (their std 1/32 sits in e4m3's
    subnormal range) and quantized to fp8; the x1024 factor on the scores
    is folded into the exp scale.  Measured rel err 6.4e-3 vs 3.2e-3 all-
    bf16 (threshold 2e-2).
  * q/k projections run as fp8 DoubleRow matmuls (lhsT [128,2,128] /
    rhs [128,2,512], K_eff=256 per step, 2x PE throughput).  v (accuracy-
    critical) stays bf16, TOKEN-major -> vaug ([128 j, 65] per j-tile per
    head: 64 v cols + a ones col that makes the AV matmul also produce
    softmax denominators).
  * scores TRANSPOSED: dotsT[j,i] = kT.T @ qT per (j-tile, head), K=64 ->
    the two heads go to PE row groups (0,0)/(64,0) and run CONCURRENTLY
    into one [128, 1024] psum tile PER J-TILE from a double-buffered
    pool: scores(jt+1) write the other buffer while the ACT exp reads
    jt's, so the PE NEVER waits on ACT for scores.  The exp uses a
    strided [2, 512-off] AP (valid columns only); diagonal tiles are
    zeroed above the diagonal with one both-heads gpsimd affine_select.
  * AV matmuls lag 2 j-tiles behind their exp so the in-order PE reaches
    them after the exp has long finished.
  * Pipeline granularity is a (batch, 512-token chunk) unit: attention
    for unit u runs interleaved (credit-weighted generator round-robin)
    with projections for unit u+1, so the PE always has independent
    matmul work queued under the exp chain.  This kills the startup
    bubble and the ACT-bound tail the batch-level pipeline had (the PE
    also drops out of its 2.4GHz p-state after ~any stall, so gaps cost
    ~2x their length).
  * The output projection of chunk c is deferred (FIFO backlog, floor 4)
    into later chunks' jt loops as PE filler; the floor keeps entries
    whose normalize stt may still be in flight out of reach of the
    in-order PE queue.  The final unit drains the backlog (its slot has
    no projection partner to interleave).
  * Normalize chain (den copy -> partition-0-staged
    reciprocal_approx_fast -> gpsimd partition_broadcast -> stt) is
    software-pipelined across the two heads.
  * Bulk DMA stays on nc.sync; the dependency-free weight loads ride the
    scalar/gpsimd queues at startup so all transfers overlap.
"""
import numpy as np
import ml_dtypes
from contextlib import ExitStack

import concourse.bacc as bacc
import concourse.mybir as mybir
import concourse.tile as tile
import concourse.bass_utils as bass_utils

F32 = mybir.dt.float32
BF16 = mybir.dt.bfloat16
FP16 = mybir.dt.float16
FP8 = mybir.dt.float8e4

B = 4            # batches
T = 2048         # tokens per batch
DIM = 1024
NT = T // 128    # token tiles per batch (16)
KT = DIM // 128  # contraction tiles (8)
NCHUNK = T // 512  # 512-col i-chunks per batch (4)
SCALE = DIM ** -0.5  # 1/32 - NOTE: full dim, not head dim (matches reference)
QK_PRESCALE = 32.0   # Wq/Wk are x32 before fp8 quantization (their std is
                     # 1/32, below e4m3's normal range); folded out of the
                     # scores inside the exp scale

TRACE = False
LAST_EXEC_NS = None
LAST_TRACE = None
LAST_PROFILE = None
_CACHED = {}


def build_kernel():
    nc = bacc.Bacc("TRN2", target_bir_lowering=False, debug=False, num_devices=8)

    xT_d = nc.dram_tensor("xt", [128, B * NCHUNK * KT * 512], BF16,
                          kind="ExternalInput").ap()
    # fp8 copy of x + 32x-prescaled fp8 Wq/Wk for the DoubleRow q/k
    # projections (contraction pairs (kt2, s): feature = (2*kt2+s)*128+p)
    x8_d = nc.dram_tensor("x8", [128, B * NCHUNK * KT * 512], FP8,
                          kind="ExternalInput").ap()
    w8q_d = nc.dram_tensor("w8q", [128, KT * 128], FP8, kind="ExternalInput").ap()
    w8k_d = nc.dram_tensor("w8k", [128, KT * 128], FP8, kind="ExternalInput").ap()
    # wv is host-pre-arranged to the SBUF layout [128, KT*128]
    # (wX[p, kt*128+m] = W[kt*128+p, m]) so its DMA is fully contiguous
    wv_d = nc.dram_tensor("wv", [128, KT * 128], BF16, kind="ExternalInput").ap()
    wo_d = nc.dram_tensor("wo", [128, DIM], BF16, kind="ExternalInput").ap()
    out_d = nc.dram_tensor("out", [B * T, DIM], FP16, kind="ExternalOutput").ap()

    xsrc = xT_d.rearrange("p (u kt t) -> p u kt t", kt=KT, t=512)
    x8src = x8_d.rearrange("p (u k2 s t) -> p u k2 s t", k2=KT // 2, s=2, t=512)

    with tile.TileContext(nc) as tc, ExitStack() as ctx:
        cp = ctx.enter_context(tc.tile_pool(name="const", bufs=1))
        xT_p = ctx.enter_context(tc.tile_pool(name="xT", bufs=2))
        x8_p = ctx.enter_context(tc.tile_pool(name="x8", bufs=2))
        qT_p = ctx.enter_context(tc.tile_pool(name="qT", bufs=2))
        kT_p = ctx.enter_context(tc.tile_pool(name="kT", bufs=2))
        vaug_p = ctx.enter_context(tc.tile_pool(name="vaug", bufs=2))
        attnT_p = ctx.enter_context(tc.tile_pool(name="attnT", bufs=6))
        recip_p = ctx.enter_context(tc.tile_pool(name="recip", bufs=4))
        avsb_p = ctx.enter_context(tc.tile_pool(name="avsb", bufs=4))  # noqa: F841 - keeps the SBUF layout of the best-measured build
        rbc_p = ctx.enter_context(tc.tile_pool(name="rbc", bufs=2))
        outT_p = ctx.enter_context(tc.tile_pool(name="outT", bufs=2))
        osb_p = ctx.enter_context(tc.tile_pool(name="osb", bufs=3))
        mm_ps = ctx.enter_context(tc.tile_pool(name="mmps", bufs=2, space="PSUM"))
        dots_ps = ctx.enter_context(tc.tile_pool(name="dotsps", bufs=2, space="PSUM"))
        av_ps_p = ctx.enter_context(tc.tile_pool(name="avps", bufs=2, space="PSUM"))

        # ---- weights.  Only wq rides the sync DMA queue (ahead of the x
        # slices); wk/wv/wo are issued from the idle vector/gpsimd/scalar
        # queues so all four transfer in parallel at startup (the issuing
        # engine only blocks on a DMA's dependencies, and weights have
        # none) ----
        w8q_sb = cp.tile([128, KT * 128], FP8, tag="w8q")
        w8k_sb = cp.tile([128, KT * 128], FP8, tag="w8k")
        wv_sb = cp.tile([128, KT * 128], BF16, tag="wv")
        wo_sb = cp.tile([128, DIM], BF16, tag="wo")
        w8qv = w8q_sb[:].rearrange("p (k2 s m) -> p k2 s m", k2=KT // 2, s=2)
        w8kv = w8k_sb[:].rearrange("p (k2 s m) -> p k2 s m", k2=KT // 2, s=2)

        def dma_weights():
            nc.scalar.dma_start(w8q_sb[:], w8q_d)
            nc.scalar.dma_start(w8k_sb[:], w8k_d)
            nc.gpsimd.dma_start(wv_sb[:], wv_d)
            nc.gpsimd.dma_start(wo_sb[:], wo_d)

        # ---- constants ----
        ones32 = cp.tile([128, NT], F32, tag="ones32")
        nc.gpsimd.memset(ones32[:], 1.0)
        onesv = ones32[:].rearrange("p (u o) -> p u o", o=1)

        state = {}     # b -> (qT, kT, vaug, outT)
        xstate = {}    # b -> xTv view (allocated 2 slices ahead of use)
        deferred = []  # (outT, t0, tt) output-projection backlog

        def ensure_xT(b2):
            if b2 not in xstate:
                xT = xT_p.tile([128, NCHUNK * KT * 512], BF16, tag="xT",
                               name="xT")
                x8 = x8_p.tile([128, NCHUNK * KT * 512], FP8, tag="x8",
                               name="x8")
                xstate[b2] = (
                    xT[:].rearrange("p (c kt t) -> p c kt t", c=NCHUNK, kt=KT),
                    x8[:].rearrange("p (c k2 s t) -> p c k2 s t",
                                    c=NCHUNK, k2=KT // 2, s=2))

        def dma_slice_u(u):
            b2, c2 = divmod(u, NCHUNK)
            if b2 >= B:
                return
            ensure_xT(b2)
            xTv, x8v = xstate[b2]
            nc.sync.dma_start(x8v[:, c2], x8src[:, u])
            nc.sync.dma_start(xTv[:, c2, :, :], xsrc[:, u, :, :])

        def emit_oproj(outT, t0, tt):
            osb = osb_p.tile([128, DIM], FP16, tag="osb", name="osb")
            for half in (0, 1):
                po = mm_ps.tile([128, 512], F32, tag="mm", name="po")
                nc.tensor.matmul(po[:], outT[:, tt * 128:(tt + 1) * 128],
                                 wo_sb[:, half * 512:(half + 1) * 512],
                                 start=True, stop=True)
                nc.vector.tensor_copy(osb[:, half * 512:(half + 1) * 512],
                                      po[:])
            nc.sync.dma_start(out_d[t0 + tt * 128: t0 + (tt + 1) * 128, :],
                              osb[:])

        def p12_chunk_steps(b, ch):
            """xT DMA + q/k (feat-major) + v (token-major) for chunk ch of
            batch b."""
            u = b * NCHUNK + ch
            if ch == 0:
                ensure_xT(b)
                qT = qT_p.tile([128, T], BF16, tag="qT", name="qT")
                kTt = kT_p.tile([128, T], BF16, tag="kT", name="kT")
                vaug = vaug_p.tile([128, NT * 130], BF16, tag="vaug", name="vaug")
                outT = outT_p.tile([128, T], BF16, tag="outT", name="outT")
                state[b] = (qT, kTt, vaug, outT)
            qT, kTt, vaug, outT = state[b]
            xTv, x8v = xstate[b]

            # keep the x DMA two chunk-slots ahead of the projections
            # (cross-batch too, so a batch's first chunk is ready on arrival)
            if u == 0:
                dma_weights()
                nc.sync.dma_start(x8v[:, 0], x8src[:, 0])
                # bf16 slice 0 (only v-proj needs it) rides the scalar
                # queue, in parallel with x8 slice 0 on sync
                nc.scalar.dma_start(xTv[:, 0, :, :], xsrc[:, 0, :, :])
                dma_slice_u(1)
            dma_slice_u(u + 2)
            yield
            # q and k projections for this 512-token chunk: fp8 DoubleRow
            # matmuls (K_eff=256 per step, 2x PE throughput); yield
            # mid-accumulation so the PE work spreads across the
            # interleaved attention pair steps
            for w8v, dest in ((w8qv, qT), (w8kv, kTt)):
                pp = mm_ps.tile([128, 512], F32, tag="mm", name="pp")
                for k2 in range(KT // 2):
                    nc.tensor.matmul(pp[:], w8v[:, k2], x8v[:, ch, k2],
                                     start=(k2 == 0), stop=(k2 == KT // 2 - 1),
                                     perf_mode=mybir.MatmulPerfMode.DoubleRow)
                    if k2 == 1:
                        yield
                nc.vector.tensor_copy(dest[:, ch * 512:(ch + 1) * 512], pp[:])
                yield
            # v for the same 4 token-tiles, token-major
            vv = vaug[:].rearrange("p (jt c) -> p jt c", c=130)
            vp = mm_ps.tile([128, 512], F32, tag="mm", name="vp")
            for j in range(4):
                for kt in range(KT):
                    nc.tensor.matmul(
                        vp[:, j * 128:(j + 1) * 128],
                        xTv[:, ch, kt, j * 128:(j + 1) * 128],
                        wv_sb[:, kt * 128:(kt + 1) * 128],
                        start=(kt == 0), stop=(kt == KT - 1))
                if j == 1:
                    yield
            src = vp[:].rearrange("p (j c) -> p j c", j=4)
            nc.vector.tensor_copy(vv[:, 4 * ch:4 * ch + 4, 0:64], src[:, :, 0:64])
            nc.vector.tensor_copy(vv[:, 4 * ch:4 * ch + 4, 65:129], src[:, :, 64:128])
            # ones columns via DVE (gpsimd's in-order queue is slow on
            # semaphore ops and would delay the normalize broadcast)
            nc.vector.tensor_copy(vv[:, 4 * ch:4 * ch + 4, 64:65],
                                  onesv[:, 4 * ch:4 * ch + 4, :])
            nc.vector.tensor_copy(vv[:, 4 * ch:4 * ch + 4, 129:130],
                                  onesv[:, 4 * ch:4 * ch + 4, :])
            yield

        def att_chunk_steps(b, c, hold=4, drain=False):
            """Attention chunk c of batch b + deferred output projections."""
            t0 = b * T
            qT, kTt, vaug, outT = state[b]
            njt = 4 * (c + 1)
            avp = {h: av_ps_p.tile([65, 512], F32, tag="av", name=f"avp{h}")
                   for h in (0, 1)}

            def emit_av(pend):
                jt, off, at = pend
                for h in (0, 1):
                    nc.tensor.matmul(
                        avp[h][:, off:512],
                        vaug[:, jt * 130 + 65 * h: jt * 130 + 65 * h + 65],
                        at[:, h * 512 + off: (h + 1) * 512],
                        start=(jt == 0), stop=(jt == njt - 1))

            # one [128,1024] dots tile PER J-TILE from a double-buffered
            # pool: scores(jt+1) write the other buffer while exp(jt) reads,
            # so the PE NEVER waits on ACT for scores (no psum WAR).  The AV
            # matmuls lag 2 j-tiles so their exp is long done when the
            # in-order PE reaches them.
            pends = []
            for jt in range(njt):
                off = max(512 * c, jt * 128) - 512 * c
                dps = dots_ps.tile([128, 1024], F32, tag="dots", name="dp")
                # h0/h1 hit disjoint PE row groups (rows 0-63 / 64-127) and
                # run concurrently
                for h in (0, 1):
                    nc.tensor.matmul(
                        dps[:, h * 512 + off: (h + 1) * 512],
                        kTt[64 * h:64 * h + 64, jt * 128:(jt + 1) * 128],
                        qT[64 * h:64 * h + 64, 512 * c + off:512 * (c + 1)],
                        start=True, stop=True)
                at = attnT_p.tile([128, 1024], BF16, tag="at", name="at")
                # exp only the valid columns of both heads (strided AP)
                dv = dps[:].rearrange("p (h i) -> p h i", h=2)
                atv = at[:].rearrange("p (h i) -> p h i", h=2)
                nc.scalar.activation(atv[:, :, off:512], dv[:, :, off:512],
                                     mybir.ActivationFunctionType.Exp,
                                     bias=0.0,
                                     scale=float(SCALE / QK_PRESCALE ** 2))
                if jt >= 4 * c:  # zero invalid (j > i) entries of the diag tile
                    # one op for both heads via the strided [2, 128] AP
                    # (halves the gpsimd op+semaphore count on the chunk tail)
                    nc.gpsimd.affine_select(
                        out=atv[:, :, off: off + 128],
                        in_=atv[:, :, off: off + 128],
                        compare_op=mybir.AluOpType.is_ge, fill=0.0,
                        base=0, pattern=[[0, 2], [1, 128]], channel_multiplier=-1)
                if len(pends) >= 2:
                    emit_av(pends.pop(0))
                # oproj backlog as PE filler.  The >4 floor keeps the
                # just-normalized chunk's entries (whose stt may still be in
                # flight - an oproj waiting on it would block the in-order
                # PE queue) out of reach; the drain unit relaxes it once its
                # own first jts are past.
                gate = hold if jt < 2 else min(hold, 0 if drain else hold)
                if len(deferred) > gate:
                    emit_oproj(*deferred.pop(0))
                pends.append((jt, off, at))
                yield
            # chunk tail: reserved oprojs interleave with the trailing AVs
            while pends:
                if deferred:
                    emit_oproj(*deferred.pop(0))
                emit_av(pends.pop(0))
            yield
            # normalize, software-pipelined across the two heads so the
            # gpsimd broadcast of h0 overlaps the DVE recip of h1 (an
            # SBUF-staged variant freed the av psum banks earlier but its
            # +20us of DVE staging measured net-slower).  NOTE: den must be
            # staged to a partition-0 tile - the custom-DVE reciprocal
            # misbehaves on mismatched partition offsets.
            rc, rb = {}, {}
            for h in (0, 1):
                den = recip_p.tile([1, 512], F32, tag="den", name="den")
                nc.vector.tensor_copy(den[:], avp[h][64:65, :])
                rc[h] = recip_p.tile([1, 512], F32, tag="recip", name="rc")
                nc.vector.reciprocal_approx_fast(rc[h][:], den[:])
                if h == 0:
                    rb[0] = rbc_p.tile([64, 512], F32, tag="rbc", name="rb")
                    nc.gpsimd.partition_broadcast(rb[0][:], rc[0][:])
            rb[1] = rbc_p.tile([64, 512], F32, tag="rbc", name="rb")
            nc.gpsimd.partition_broadcast(rb[1][:], rc[1][:])
            for h in (0, 1):
                nc.vector.scalar_tensor_tensor(
                    outT[64 * h:64 * h + 64, c * 512:(c + 1) * 512],
                    avp[h][0:64, :], 1.0, rb[h][:],
                    op0=mybir.AluOpType.mult, op1=mybir.AluOpType.mult)
            deferred.extend((outT, t0, tt) for tt in range(4 * c, 4 * c + 4))
            yield

        def drive(gens):
            """Credit-weighted round-robin of (generator, weight) pairs: a
            generator advances ~weight steps per round, so both exhaust at
            the same time and the emission (= in-order engine queue order)
            keeps independent work spread between the dependency chains."""
            gens = [gw for gw in gens if gw is not None and gw[0] is not None]
            credit = [0.0] * len(gens)
            alive = [True] * len(gens)
            while any(alive):
                for i, (g, w) in enumerate(gens):
                    if not alive[i]:
                        continue
                    credit[i] += w
                    while credit[i] >= 1.0 and alive[i]:
                        credit[i] -= 1.0
                        try:
                            next(g)
                        except StopIteration:
                            alive[i] = False

        # oproj backlog floor per attention unit: steady 4 (defers each
        # chunk's oproj ~2 chunks), hoard through the penultimate units so
        # the ACT-bound final unit (no p12 partner) has PE filler, then
        # drain it there (hold 0)
        DRAIN_UNIT = (B - 1, NCHUNK - 1)  # last unit: ACT-bound, no p12
        # partner - drain the whole oproj backlog through its jt loop
        P12_STEPS = 7.0
        units = [(b, ch) for b in range(B) for ch in range(NCHUNK)]
        prev = None
        for i, (b, ch) in enumerate(units):
            att = (att_chunk_steps(*prev, drain=(prev == DRAIN_UNIT))
                   if prev is not None else None)
            att_steps = 4 * (prev[1] + 1) + 2 if prev is not None else 1
            drive([(att, 1.0) if att is not None else None,
                   (p12_chunk_steps(b, ch),
                    P12_STEPS / att_steps if att is not None else 4.0)])
            prev = (b, ch)
        drive([(att_chunk_steps(*prev, drain=(prev == DRAIN_UNIT)), 1.0)])
        while deferred:
            emit_oproj(*deferred.pop(0))

    nc.compile()
    return nc


def kernel(x, Wq, Wkv, Wout, bout):
    """Full inputs -> full output. Shards across 8 NeuronCores internally."""
    global LAST_EXEC_NS, LAST_TRACE
    if "nc" not in _CACHED:
        _CACHED["nc"] = build_kernel()
    nc = _CACHED["nc"]

    hdt = ml_dtypes.bfloat16
    f8 = ml_dtypes.float8_e4m3
    xf = np.asarray(x, dtype=np.float32).reshape(B, NCHUNK, 512, DIM)
    # [128, B, NCHUNK, KT, 512]: per-(chunk,partition) rows are 8KB
    # contiguous in DRAM so the per-chunk DMA moves full-size packets
    xTf = (xf.transpose(3, 0, 1, 2)         # [DIM, B, NCHUNK, 512]
             .reshape(KT, 128, B, NCHUNK, 512)
             .transpose(1, 2, 3, 0, 4))
    xT = np.ascontiguousarray(xTf).astype(hdt).reshape(128, -1)
    # fp8 copy with the kt dim regrouped into DoubleRow (kt2, s) pairs:
    # feature = (2*kt2+s)*128+p, laid out [p, b, ch, kt2, s, t]
    x8 = np.ascontiguousarray(xTf).astype(f8).reshape(128, -1)
    Wq = np.asarray(Wq, dtype=np.float32)
    Wkv = np.asarray(Wkv, dtype=np.float32)
    Wout = np.asarray(Wout, dtype=np.float32).astype(hdt)
    bout = np.asarray(bout, dtype=np.float32)

    def wlayout(w):  # [DIM, 128] -> [128, KT*128] SBUF layout for clean DMA
        return np.ascontiguousarray(
            w.reshape(KT, 128, 128).transpose(1, 0, 2).reshape(128, KT * 128))

    def wlayout8(w):  # same, x32 prescale, fp8
        return wlayout(np.asarray(w) * QK_PRESCALE).astype(f8)

    in_maps = []
    for c in range(8):
        s = slice(128 * c, 128 * (c + 1))
        in_maps.append({
            "xt": xT,
            "x8": x8,
            "w8q": wlayout8(Wq[:, s]),
            "w8k": wlayout8(Wkv[:, :DIM][:, s]),
            "wv": wlayout(Wkv[:, DIM:][:, s].astype(hdt)),
            "wo": np.ascontiguousarray(Wout[s, :]),
        })

    res = bass_utils.run_bass_kernel_spmd(nc, in_maps, core_ids=list(range(8)),
                                          trace=TRACE)
    if TRACE:
        LAST_EXEC_NS = res.exec_time_ns
        LAST_TRACE = res.instructions_and_trace
        globals()["LAST_PROFILE"] = getattr(res, "profile_json", None)
    acc = res.results[0]["out"].astype(np.float64)
    for c in range(1, 8):
        acc += res.results[c]["out"]
    out = (acc + bout.astype(np.float64)).astype(np.float32)
    return out.reshape(B, T, DIM)
